# revision 23
# baseline (speedup 1.0000x reference)
"""MoE layer (nn_MoELayer_4681514353281) Trainium2 Bass kernel — expert-paired.

Same sparse-routing scheme as kernel.py (host gate, packed selected tokens,
bf16 FFN with LoRA folded), but each core carries TWO experts at quarter-F
width instead of one expert at half-F: pair a high-count expert with a
low-count one so the per-core token total is balanced (~4092) instead of
every core paying the max expert count (2×2097). 8 cores = 2 pairs x 4
F-quarters. Also removes one ragged down-chunk and one LDWEIGHTS-floored
tail up-block per core.
"""

import os
import sys

sys.path.insert(0, "/opt/trn_rl_repo")

import ml_dtypes
import numpy as np

B, S, D, F, E, R = 2, 4096, 1024, 4096, 16, 16
TOPK = 4
N_TOK = B * S
FQ = F // 4            # 1024 per-core F quarter
TOK_BLK = 512
DC = D // 128          # 8
FC = FQ // 128         # 8

BF16NP = ml_dtypes.bfloat16

_programs = {}
LAST_RESULTS = None


def _plan_blocks(t_exact):
    blocks = []
    t0 = 0
    while t_exact - t0 > TOK_BLK:
        blocks.append((t0, TOK_BLK))
        t0 += TOK_BLK
    if t_exact > t0:
        blocks.append((t0, t_exact - t0))
    return blocks


def _build_program(ta, tb):
    import concourse.tile as tile
    from concourse import bacc, mybir

    F32 = mybir.dt.float32
    BF16 = mybir.dt.bfloat16
    AF = mybir.ActivationFunctionType

    segs = []
    for name, t_exact in (("a", ta), ("b", tb)):
        segs.append({
            "name": name,
            "t": t_exact,
            "n128": -(-t_exact // 128),
            "blocks": _plan_blocks(t_exact),
        })

    nc = bacc.Bacc("TRN2", target_bir_lowering=False, debug=False, num_devices=8)

    for s in segs:
        nm = s["name"]
        s["xTd"] = nc.dram_tensor(f"xT{nm}", [D, s["t"]], BF16, kind="ExternalInput")
        s["w1d"] = nc.dram_tensor(f"w1{nm}", [D, FQ], BF16, kind="ExternalInput")
        s["w2d"] = nc.dram_tensor(f"w2{nm}", [FQ, D], BF16, kind="ExternalInput")
        s["wcd"] = nc.dram_tensor(f"wc{nm}", [128, s["n128"]], F32,
                                  kind="ExternalInput")
        s["outd"] = nc.dram_tensor(f"out{nm}", [s["t"], D], F32,
                                   kind="ExternalOutput")

    with tile.TileContext(nc) as tc:
        with (
            tc.tile_pool(name="singles", bufs=1) as singles,
            tc.tile_pool(name="xp", bufs=3) as xp,
            tc.tile_pool(name="hap", bufs=FC + 2) as hap,
            tc.tile_pool(name="outp", bufs=3) as outp,
            tc.tile_pool(name="psH", bufs=3, space="PSUM") as psH,
            tc.tile_pool(name="psEO", bufs=5, space="PSUM") as psEO,
        ):
            for s in segs:
                nm = s["name"]
                s["w1"] = singles.tile([128, FC, DC, 128], BF16,
                                       name=f"w1t{nm}")
                s["w2"] = singles.tile([128, FC, D], BF16, name=f"w2t{nm}")
                s["w_all"] = singles.tile([128, s["n128"]], F32,
                                          name=f"wct{nm}")
                s["xT_r"] = s["xTd"].rearrange("(dc p) t -> p dc t", p=128)
                s["w1_r"] = s["w1d"].rearrange("(dc p) (fc q) -> p fc dc q",
                                               p=128, q=128)
                s["w2_r"] = s["w2d"].rearrange("(fc p) d -> p fc d", p=128)

            def load_block(s, t0, w, eng=None):
                t = xp.tile([128, DC, TOK_BLK], BF16, tag="xb")
                (eng or nc.sync).dma_start(t[:, :, :w], s["xT_r"][:, :, t0:t0 + w])
                return t

            # PE clock warmup over the DMA head
            zw = singles.tile([128, 640], BF16)
            nc.vector.memset(zw[:], 0.0)
            psw = psH.tile([128, TOK_BLK], F32, tag="ph")
            for _ in range(10):
                nc.tensor.matmul(psw[:], zw[:, :128], zw[:, 128:640],
                                 start=True, stop=True)

            # Head: x block 0 rides the (idle) scalar queue in parallel with
            # the weight stream on sync — the two queues coexist at
            # ~115+210GB/s, so the PE-start critical path is max(x0, w1 f0)
            # instead of their sum. Everything else stays on sync in
            # need-order.
            sa, sb = segs
            # x block 0 rides gpsimd alone: the arbiter prioritizes it, and
            # the scalar queue's head is handicapped by the framework's
            # ACT_TABLE_LOADs. Weights stream on sync concurrently, so the
            # PE-start critical path is max(x0, w1 f0), not their sum.
            sa["xb"] = load_block(sa, *sa["blocks"][0], eng=nc.gpsimd)
            for fc in range(FC):
                nc.sync.dma_start(sa["w1"][:, fc, :, :], sa["w1_r"][:, fc, :, :])
            nc.sync.dma_start(sa["w_all"][:], sa["wcd"][:, :])
            for fc in range(FC // 2):
                nc.sync.dma_start(sa["w2"][:, fc, :], sa["w2_r"][:, fc, :])

            for si, s in enumerate(segs):
                blocks = s["blocks"]
                w1, w2, w_all = s["w1"], s["w2"], s["w_all"]
                xb = s["xb"] if si == 0 else s.pop("xb_pre")
                for blk, (t0, w) in enumerate(blocks):
                    h_all = []
                    for fc in range(FC):
                        ps_h = psH.tile([128, TOK_BLK], F32, tag="ph")
                        for dc in range(DC):
                            nc.tensor.matmul(
                                ps_h[:, :w], w1[:, fc, dc, :], xb[:, dc, :w],
                                start=(dc == 0), stop=(dc == DC - 1),
                            )
                        h = hap.tile([128, TOK_BLK], BF16, tag="h")
                        nc.scalar.activation(h[:, :w], ps_h[:, :w],
                                             AF.Gelu_apprx_tanh)
                        h_all.append(h)

                    if blk + 1 < len(blocks):
                        xb_next = load_block(s, *blocks[blk + 1])
                    else:
                        xb_next = None
                    if si == 0 and blk == 0:
                        # rest of segA w2, then the whole segB stream queues
                        # behind it (consumed ~110us later)
                        for fc in range(FC // 2, FC):
                            nc.sync.dma_start(w2[:, fc, :], s["w2_r"][:, fc, :])
                        for fc in range(FC):
                            nc.sync.dma_start(sb["w1"][:, fc, :, :],
                                              sb["w1_r"][:, fc, :, :])
                        sb["xb_pre"] = load_block(sb, *sb["blocks"][0])
                        nc.sync.dma_start(sb["w_all"][:], sb["wcd"][:, :])
                        for fc in range(FC):
                            nc.sync.dma_start(sb["w2"][:, fc, :],
                                              sb["w2_r"][:, fc, :])

                    subs = []
                    off = 0
                    while off < w:
                        subs.append((off, min(128, w - off)))
                        off += 128
                    for off, cw in subs:
                        ob = outp.tile([128, D], F32, tag="ob")
                        col = (t0 + off) // 128
                        trow = t0 + off
                        last = (si == 1 and blk == len(blocks) - 1
                                and (off, cw) == subs[-1])
                        for dh in range(2):
                            eo = psEO.tile([128, 512], F32, tag="eo")
                            for fc in range(FC):
                                nc.tensor.matmul(
                                    eo[:cw, :],
                                    h_all[fc][:, off:off + cw],
                                    w2[:, fc, dh * 512:(dh + 1) * 512],
                                    start=(fc == 0), stop=(fc == FC - 1),
                                )
                            nc.vector.tensor_scalar_mul(
                                ob[:cw, dh * 512:(dh + 1) * 512], eo[:cw, :],
                                scalar1=w_all[:cw, col:col + 1],
                            )
                            if last:
                                nc.scalar.dma_start(
                                    s["outd"][trow:trow + cw,
                                              dh * 512:(dh + 1) * 512],
                                    ob[:cw, dh * 512:(dh + 1) * 512],
                                )
                        if not last:
                            nc.scalar.dma_start(s["outd"][trow:trow + cw, :],
                                                ob[:cw, :])

                    xb = xb_next

    nc.compile()
    return nc


def _get_program(ta, tb):
    key = (ta, tb)
    if key not in _programs:
        _programs[key] = _build_program(ta, tb)
    return _programs[key]


def _gate_weights(x2d, Wg):
    try:
        import jax
        import jax.numpy as jnp
        cpu = jax.devices("cpu")[0]
        with jax.default_device(cpu):
            xf = jnp.asarray(x2d, jnp.float32)
            wg = jnp.asarray(Wg, jnp.float32)
            weights = jax.nn.softmax(xf @ wg, axis=-1)
            top_w, top_idx = jax.lax.top_k(weights, TOPK)
            top_w = top_w / jnp.sum(top_w, axis=-1, keepdims=True)
            cols = [jnp.sum(top_w * (top_idx == i), axis=-1) for i in range(TOPK)]
            return np.asarray(jnp.stack(cols, axis=-1), np.float32)
    except Exception:
        logits = x2d.astype(np.float32) @ Wg.astype(np.float32)
        m = logits.max(axis=-1, keepdims=True)
        e = np.exp((logits - m).astype(np.float32), dtype=np.float32)
        p = (e / e.sum(axis=-1, keepdims=True).astype(np.float32)).astype(np.float32)
        idx = np.argsort(-p, axis=-1, kind="stable")[:, :TOPK]
        topw = np.take_along_axis(p, idx, axis=-1)
        topw = (topw / topw.sum(axis=-1, keepdims=True)).astype(np.float32)
        w = np.zeros((x2d.shape[0], TOPK), np.float32)
        for i in range(TOPK):
            w[:, i] = (topw * (idx == i)).sum(axis=-1)
        return w


def kernel(x, Wg, W1, A1, B1, W2, A2, B2):
    global LAST_RESULTS
    from concourse.bass_utils import run_bass_kernel_spmd

    x = np.asarray(x, dtype=np.float32)
    x2d = x.reshape(N_TOK, D)
    w4 = _gate_weights(x2d, np.asarray(Wg, dtype=np.float32))

    idx = [np.nonzero(w4[:, e] > 0)[0] for e in range(TOPK)]
    counts = [len(ix) for ix in idx]
    # pair the largest-count expert with the smallest, second with third
    order = sorted(range(TOPK), key=lambda e: -counts[e])
    pairs = [(order[0], order[3]), (order[1], order[2])]
    ta = max(128, max(counts[a] for a, _ in pairs))
    tb = max(128, max(counts[b] for _, b in pairs))

    nc = _get_program(ta, tb)

    def pack_x(e, t_pad):
        ce = counts[e]
        xpck = np.zeros((t_pad, D), dtype=BF16NP)
        xpck[:ce] = x2d[idx[e]]
        return np.ascontiguousarray(xpck.T)

    def pack_wc(e, t_pad):
        n128 = -(-t_pad // 128)
        wc = np.zeros(n128 * 128, dtype=np.float32)
        wc[:counts[e]] = w4[idx[e], e]
        return np.ascontiguousarray(wc.reshape(n128, 128).T)

    def fold(e):
        w1c = (np.asarray(W1[e], np.float64)
               + np.asarray(A1[e], np.float64) @ np.asarray(B1[e], np.float64))
        w2c = (np.asarray(W2[e], np.float64)
               + np.asarray(A2[e], np.float64) @ np.asarray(B2[e], np.float64))
        return w1c.astype(BF16NP), w2c.astype(BF16NP)

    folded = {}
    for e in set(e for p in pairs for e in p):
        folded[e] = fold(e)

    in_maps = []
    for core in range(8):
        pi, q = core // 4, core % 4
        eA, eB = pairs[pi]
        f0, f1 = q * FQ, (q + 1) * FQ
        m = {}
        for nm, e, t_pad in (("a", eA, ta), ("b", eB, tb)):
            w1c, w2c = folded[e]
            m[f"xT{nm}"] = pack_x(e, t_pad)
            m[f"w1{nm}"] = np.ascontiguousarray(w1c[:, f0:f1])
            m[f"w2{nm}"] = np.ascontiguousarray(w2c[f0:f1, :])
            m[f"wc{nm}"] = pack_wc(e, t_pad)
        in_maps.append(m)

    trace = bool(os.environ.get("KERNEL_TRACE"))
    res = None
    last_exc = None
    for attempt in range(3):
        try:
            res = run_bass_kernel_spmd(
                nc, in_maps, core_ids=list(range(8)), trace=trace
            )
            break
        except Exception as exc:
            last_exc = exc
            if attempt >= 1:
                trace = False
    if res is None:
        raise last_exc
    LAST_RESULTS = res

    acc = np.zeros((N_TOK, D), dtype=np.float32)
    for pi, (eA, eB) in enumerate(pairs):
        for nm, e in (("a", eA), ("b", eB)):
            ce = counts[e]
            tot = np.zeros((ce, D), dtype=np.float32)
            for q in range(4):
                tot += np.asarray(res.results[pi * 4 + q][f"out{nm}"][:ce],
                                  np.float32)
            acc[idx[e]] += tot
    return acc.reshape(B, S, D)


# revision 24
# speedup vs baseline: 1.0084x; 1.0084x over previous
"""MoE layer (nn_MoELayer_4681514353281) Trainium2 Bass kernel — expert-paired.

Same sparse-routing scheme as kernel.py (host gate, packed selected tokens,
bf16 FFN with LoRA folded), but each core carries TWO experts at quarter-F
width instead of one expert at half-F: pair a high-count expert with a
low-count one so the per-core token total is balanced (~4092) instead of
every core paying the max expert count (2×2097). 8 cores = 2 pairs x 4
F-quarters. Also removes one ragged down-chunk and one LDWEIGHTS-floored
tail up-block per core.
"""

import os
import sys

sys.path.insert(0, "/opt/trn_rl_repo")

import ml_dtypes
import numpy as np

B, S, D, F, E, R = 2, 4096, 1024, 4096, 16, 16
TOPK = 4
N_TOK = B * S
FQ = F // 4            # 1024 per-core F quarter
TOK_BLK = 512
DC = D // 128          # 8
FC = FQ // 128         # 8

BF16NP = ml_dtypes.bfloat16

_programs = {}
LAST_RESULTS = None


def _plan_blocks(t_exact):
    blocks = []
    t0 = 0
    while t_exact - t0 > TOK_BLK:
        blocks.append((t0, TOK_BLK))
        t0 += TOK_BLK
    if t_exact > t0:
        blocks.append((t0, t_exact - t0))
    return blocks


def _build_program(ta, tb):
    import concourse.tile as tile
    from concourse import bacc, mybir

    F32 = mybir.dt.float32
    BF16 = mybir.dt.bfloat16
    AF = mybir.ActivationFunctionType

    segs = []
    for name, t_exact in (("a", ta), ("b", tb)):
        segs.append({
            "name": name,
            "t": t_exact,
            "n128": -(-t_exact // 128),
            "blocks": _plan_blocks(t_exact),
        })

    nc = bacc.Bacc("TRN2", target_bir_lowering=False, debug=False, num_devices=8)

    for s in segs:
        nm = s["name"]
        s["xTd"] = nc.dram_tensor(f"xT{nm}", [D, s["t"]], BF16, kind="ExternalInput")
        s["w1d"] = nc.dram_tensor(f"w1{nm}", [D, FQ], BF16, kind="ExternalInput")
        s["w2d"] = nc.dram_tensor(f"w2{nm}", [FQ, D], BF16, kind="ExternalInput")
        s["wcd"] = nc.dram_tensor(f"wc{nm}", [128, s["n128"]], F32,
                                  kind="ExternalInput")
        s["outd"] = nc.dram_tensor(f"out{nm}", [s["t"], D], F32,
                                   kind="ExternalOutput")

    with tile.TileContext(nc) as tc:
        with (
            tc.tile_pool(name="singles", bufs=1) as singles,
            tc.tile_pool(name="xp", bufs=3) as xp,
            tc.tile_pool(name="hap", bufs=FC + 2) as hap,
            tc.tile_pool(name="outp", bufs=3) as outp,
            tc.tile_pool(name="psH", bufs=3, space="PSUM") as psH,
            tc.tile_pool(name="psEO", bufs=5, space="PSUM") as psEO,
        ):
            for s in segs:
                nm = s["name"]
                s["w1"] = singles.tile([128, FC, DC, 128], BF16,
                                       name=f"w1t{nm}")
                s["w2"] = singles.tile([128, FC, D], BF16, name=f"w2t{nm}")
                s["w_all"] = singles.tile([128, s["n128"]], F32,
                                          name=f"wct{nm}")
                s["xT_r"] = s["xTd"].rearrange("(dc p) t -> p dc t", p=128)
                s["w1_r"] = s["w1d"].rearrange("(dc p) (fc q) -> p fc dc q",
                                               p=128, q=128)
                s["w2_r"] = s["w2d"].rearrange("(fc p) d -> p fc d", p=128)

            def load_block(s, t0, w, eng=None):
                t = xp.tile([128, DC, TOK_BLK], BF16, tag="xb")
                (eng or nc.sync).dma_start(t[:, :, :w], s["xT_r"][:, :, t0:t0 + w])
                return t

            # PE clock warmup over the DMA head
            zw = singles.tile([128, 640], BF16)
            nc.vector.memset(zw[:], 0.0)
            psw = psH.tile([128, TOK_BLK], F32, tag="ph")
            for _ in range(10):
                nc.tensor.matmul(psw[:], zw[:, :128], zw[:, 128:640],
                                 start=True, stop=True)

            # Head: x block 0 rides the (idle) scalar queue in parallel with
            # the weight stream on sync — the two queues coexist at
            # ~115+210GB/s, so the PE-start critical path is max(x0, w1 f0)
            # instead of their sum. Everything else stays on sync in
            # need-order.
            sa, sb = segs
            t0a, wa = sa["blocks"][0]
            xa0 = xp.tile([128, DC, TOK_BLK], BF16, tag="xb")
            hd = DC // 2
            nc.scalar.dma_start(xa0[:, :hd, :wa], sa["xT_r"][:, :hd, t0a:t0a + wa])
            nc.gpsimd.dma_start(xa0[:, hd:, :wa], sa["xT_r"][:, hd:, t0a:t0a + wa])
            sa["xb"] = xa0
            for fc in range(FC):
                nc.sync.dma_start(sa["w1"][:, fc, :, :], sa["w1_r"][:, fc, :, :])
            nc.sync.dma_start(sa["w_all"][:], sa["wcd"][:, :])
            for fc in range(FC // 2):
                nc.sync.dma_start(sa["w2"][:, fc, :], sa["w2_r"][:, fc, :])

            for si, s in enumerate(segs):
                blocks = s["blocks"]
                w1, w2, w_all = s["w1"], s["w2"], s["w_all"]
                xb = s["xb"] if si == 0 else s.pop("xb_pre")
                for blk, (t0, w) in enumerate(blocks):
                    h_all = []
                    for fc in range(FC):
                        ps_h = psH.tile([128, TOK_BLK], F32, tag="ph")
                        for dc in range(DC):
                            nc.tensor.matmul(
                                ps_h[:, :w], w1[:, fc, dc, :], xb[:, dc, :w],
                                start=(dc == 0), stop=(dc == DC - 1),
                            )
                        h = hap.tile([128, TOK_BLK], BF16, tag="h")
                        nc.scalar.activation(h[:, :w], ps_h[:, :w],
                                             AF.Gelu_apprx_tanh)
                        h_all.append(h)

                    if blk + 1 < len(blocks):
                        xb_next = load_block(s, *blocks[blk + 1])
                    else:
                        xb_next = None
                    if si == 0 and blk == 0:
                        # rest of segA w2, then the whole segB stream queues
                        # behind it (consumed ~110us later)
                        for fc in range(FC // 2, FC):
                            nc.sync.dma_start(w2[:, fc, :], s["w2_r"][:, fc, :])
                        for fc in range(FC):
                            nc.sync.dma_start(sb["w1"][:, fc, :, :],
                                              sb["w1_r"][:, fc, :, :])
                        sb["xb_pre"] = load_block(sb, *sb["blocks"][0])
                        nc.sync.dma_start(sb["w_all"][:], sb["wcd"][:, :])
                        for fc in range(FC):
                            nc.sync.dma_start(sb["w2"][:, fc, :],
                                              sb["w2_r"][:, fc, :])

                    subs = []
                    off = 0
                    while off < w:
                        subs.append((off, min(128, w - off)))
                        off += 128
                    for off, cw in subs:
                        ob = outp.tile([128, D], F32, tag="ob")
                        col = (t0 + off) // 128
                        trow = t0 + off
                        last = (si == 1 and blk == len(blocks) - 1
                                and (off, cw) == subs[-1])
                        for dh in range(2):
                            eo = psEO.tile([128, 512], F32, tag="eo")
                            for fc in range(FC):
                                nc.tensor.matmul(
                                    eo[:cw, :],
                                    h_all[fc][:, off:off + cw],
                                    w2[:, fc, dh * 512:(dh + 1) * 512],
                                    start=(fc == 0), stop=(fc == FC - 1),
                                )
                            nc.vector.tensor_scalar_mul(
                                ob[:cw, dh * 512:(dh + 1) * 512], eo[:cw, :],
                                scalar1=w_all[:cw, col:col + 1],
                            )
                            if last:
                                nc.scalar.dma_start(
                                    s["outd"][trow:trow + cw,
                                              dh * 512:(dh + 1) * 512],
                                    ob[:cw, dh * 512:(dh + 1) * 512],
                                )
                        if not last:
                            nc.scalar.dma_start(s["outd"][trow:trow + cw, :],
                                                ob[:cw, :])

                    xb = xb_next

    nc.compile()
    return nc


def _get_program(ta, tb):
    key = (ta, tb)
    if key not in _programs:
        _programs[key] = _build_program(ta, tb)
    return _programs[key]


def _gate_weights(x2d, Wg):
    try:
        import jax
        import jax.numpy as jnp
        cpu = jax.devices("cpu")[0]
        with jax.default_device(cpu):
            xf = jnp.asarray(x2d, jnp.float32)
            wg = jnp.asarray(Wg, jnp.float32)
            weights = jax.nn.softmax(xf @ wg, axis=-1)
            top_w, top_idx = jax.lax.top_k(weights, TOPK)
            top_w = top_w / jnp.sum(top_w, axis=-1, keepdims=True)
            cols = [jnp.sum(top_w * (top_idx == i), axis=-1) for i in range(TOPK)]
            return np.asarray(jnp.stack(cols, axis=-1), np.float32)
    except Exception:
        logits = x2d.astype(np.float32) @ Wg.astype(np.float32)
        m = logits.max(axis=-1, keepdims=True)
        e = np.exp((logits - m).astype(np.float32), dtype=np.float32)
        p = (e / e.sum(axis=-1, keepdims=True).astype(np.float32)).astype(np.float32)
        idx = np.argsort(-p, axis=-1, kind="stable")[:, :TOPK]
        topw = np.take_along_axis(p, idx, axis=-1)
        topw = (topw / topw.sum(axis=-1, keepdims=True)).astype(np.float32)
        w = np.zeros((x2d.shape[0], TOPK), np.float32)
        for i in range(TOPK):
            w[:, i] = (topw * (idx == i)).sum(axis=-1)
        return w


def kernel(x, Wg, W1, A1, B1, W2, A2, B2):
    global LAST_RESULTS
    from concourse.bass_utils import run_bass_kernel_spmd

    x = np.asarray(x, dtype=np.float32)
    x2d = x.reshape(N_TOK, D)
    w4 = _gate_weights(x2d, np.asarray(Wg, dtype=np.float32))

    idx = [np.nonzero(w4[:, e] > 0)[0] for e in range(TOPK)]
    counts = [len(ix) for ix in idx]
    # pair the largest-count expert with the smallest, second with third
    order = sorted(range(TOPK), key=lambda e: -counts[e])
    pairs = [(order[0], order[3]), (order[1], order[2])]
    ta = max(128, max(counts[a] for a, _ in pairs))
    tb = max(128, max(counts[b] for _, b in pairs))

    nc = _get_program(ta, tb)

    def pack_x(e, t_pad):
        ce = counts[e]
        xpck = np.zeros((t_pad, D), dtype=BF16NP)
        xpck[:ce] = x2d[idx[e]]
        return np.ascontiguousarray(xpck.T)

    def pack_wc(e, t_pad):
        n128 = -(-t_pad // 128)
        wc = np.zeros(n128 * 128, dtype=np.float32)
        wc[:counts[e]] = w4[idx[e], e]
        return np.ascontiguousarray(wc.reshape(n128, 128).T)

    def fold(e):
        w1c = (np.asarray(W1[e], np.float64)
               + np.asarray(A1[e], np.float64) @ np.asarray(B1[e], np.float64))
        w2c = (np.asarray(W2[e], np.float64)
               + np.asarray(A2[e], np.float64) @ np.asarray(B2[e], np.float64))
        return w1c.astype(BF16NP), w2c.astype(BF16NP)

    folded = {}
    for e in set(e for p in pairs for e in p):
        folded[e] = fold(e)

    in_maps = []
    for core in range(8):
        pi, q = core // 4, core % 4
        eA, eB = pairs[pi]
        f0, f1 = q * FQ, (q + 1) * FQ
        m = {}
        for nm, e, t_pad in (("a", eA, ta), ("b", eB, tb)):
            w1c, w2c = folded[e]
            m[f"xT{nm}"] = pack_x(e, t_pad)
            m[f"w1{nm}"] = np.ascontiguousarray(w1c[:, f0:f1])
            m[f"w2{nm}"] = np.ascontiguousarray(w2c[f0:f1, :])
            m[f"wc{nm}"] = pack_wc(e, t_pad)
        in_maps.append(m)

    trace = bool(os.environ.get("KERNEL_TRACE"))
    res = None
    last_exc = None
    for attempt in range(3):
        try:
            res = run_bass_kernel_spmd(
                nc, in_maps, core_ids=list(range(8)), trace=trace
            )
            break
        except Exception as exc:
            last_exc = exc
            if attempt >= 1:
                trace = False
    if res is None:
        raise last_exc
    LAST_RESULTS = res

    acc = np.zeros((N_TOK, D), dtype=np.float32)
    for pi, (eA, eB) in enumerate(pairs):
        for nm, e in (("a", eA), ("b", eB)):
            ce = counts[e]
            tot = np.zeros((ce, D), dtype=np.float32)
            for q in range(4):
                tot += np.asarray(res.results[pi * 4 + q][f"out{nm}"][:ce],
                                  np.float32)
            acc[idx[e]] += tot
    return acc.reshape(B, S, D)
